# revision 51
# baseline (speedup 1.0000x reference)
"""Grok1-style MoE (T=2048, H=1024, E=8, I=2048, top-2) on 8 Trainium2 cores.

Strategy (expert-parallel, per the sharding hint):
  - Host: compute the tiny router (x @ gate_w, tanh softcap, top-2, softmax)
    and dispatch tokens by expert assignment (the "all-to-all dispatch" step:
    with full inputs on the host, dispatch = gather per expert), packing the
    per-core shards in the device-friendly tiled layout.
  - Device (SPMD, 1 expert per core), all GEMMs in bf16 with fp32 PSUM
    accumulation (bf16 enables fast weight load and halves HBM traffic;
    end-to-end rel err ~4e-3):
      Phase 1:  gT = wg_e^T x_e^T ; uT = wu_e^T x_e^T      ([I, M], M moving)
                act = gelu_tanh(gT) * uT                    (stored bf16)
      Phase 2:  yT[h, m] = sum_i wd_e[i, h] * act[i, m]     ([H, M], M moving)
                y = yT * probs (broadcast over h)           (row-scaled)
    Tokens ride in the matmul free dim in BOTH phases, so M needs no
    128-padding — M_PAD is max_e |tokens_e| rounded up to chunk granularity.
  - Host: combine = scatter-add per-expert outputs into [T, H].

Perf notes: x is pre-chunked on the host so every DMA is contiguous per
partition (128 descriptors instead of 1024 — HWDGE descriptor generation at
~10ns/desc was on the startup critical path); a run of zero matmuls at the
top warms the PE HAM clock gate (1.2 -> 2.4 GHz) during the startup DMA
window so real matmuls start at full clock.
"""

import numpy as np
import ml_dtypes

import concourse.mybir as mybir
import concourse.tile as tile
from concourse import bacc
from concourse.bass_utils import run_bass_kernel_spmd

T, H, E, I_DIM, TOPK = 2048, 1024, 8, 2048, 2
SOFTCAP = 30.0
P = 128
N_CORES = 8
KH = H // P      # 8 contraction tiles (phase 1)
NI = I_DIM // P  # 16 i tiles
NHT = H // P     # 8 h tiles (phase 2 output partition tiles)
# Zero-matmuls to warm the HAM clock gate. The 4096-cycle activity window is
# free-running, so the 1.2->2.4 GHz flip needs a window-ALIGNED fully-busy
# span: with only ~3.4us of warmup the flip lands ~12us into the real stream
# (measured: first-decile matmuls at 151ns vs 115ns warm, ~2.8us lost).
# 30 warmups (~6us) keep the PE busy until the first matmul's DMA deps
# land (measured posting at 13.4-14.5us), so the flip happens before the
# real stream starts and the engine never idles at the handoff.
N_WARM = 30

BF16 = ml_dtypes.bfloat16

_compiled = {}
LAST_RESULTS = None


def _chunk_shape(n_max):
    """M_PAD = NCH equal chunks of MC (<=512, mult of 4) covering n_max."""
    nch = max(1, -(-n_max // 512))
    mc = -(-n_max // (nch * 4)) * 4
    return nch, mc


def _build(NCH, MC):
    M_PAD = NCH * MC
    f32 = mybir.dt.float32
    bf16 = mybir.dt.bfloat16

    nc = bacc.Bacc("TRN2", target_bir_lowering=False, num_devices=N_CORES)
    # Host-packed layouts (all DMAs contiguous per partition):
    #   xt    [NCH, P, KH, MC]: xt[c, p, k, j] = x_e[c*MC+j, k*P+p]
    #   wg    [NI, P, KH*P]   : wg[it, p, k*P+i] = wg_e[k*P+p, it*P+i]
    #   wu    same as wg
    #   wd    [NI, P, H]      : wd[it, p, h] = wd_e[it*P+p, h]
    #   probs [P, M_PAD]      : prob broadcast across partitions
    #   y     [NHT, P, M_PAD] : y[ht, p, m] = out_e[m, ht*P+p] (pre-transpose)
    xt = nc.dram_tensor("xt", [NCH, P, KH, MC], bf16, kind="ExternalInput")
    wg = nc.dram_tensor("wg", [NI, P, KH * P], bf16, kind="ExternalInput")
    wu = nc.dram_tensor("wu", [NI, P, KH * P], bf16, kind="ExternalInput")
    wd = nc.dram_tensor("wd", [NI, P, H], bf16, kind="ExternalInput")
    probs = nc.dram_tensor("probs", [P, M_PAD], f32, kind="ExternalInput")
    y = nc.dram_tensor("y", [NHT, P, M_PAD], f32, kind="ExternalOutput")

    with tile.TileContext(nc) as tc:
        with (
            tc.tile_pool(name="persist", bufs=1) as persist,
            tc.tile_pool(name="warm", bufs=1) as warm,
            tc.tile_pool(name="wtiles", bufs=6) as wtiles,
            tc.tile_pool(name="gtmps", bufs=3) as gtmps,
            tc.tile_pool(name="outs", bufs=3) as outs,
            tc.tile_pool(name="psum", bufs=2, space="PSUM") as psum,
            tc.tile_pool(name="wpsum", bufs=1, space="PSUM") as wpsum,
        ):
            # per-chunk xt tiles so the first matmul only waits on chunk 0
            xt_sb = [persist.tile([P, KH, MC], bf16, name=f"xt{c}")
                     for c in range(NCH)]
            probs_sb = persist.tile([P, M_PAD], f32)
            wd_sb = persist.tile([P, NI, H], bf16)
            acts = persist.tile([P, NI, M_PAD], bf16)

            # HAM warm-up: keep the PE busy on zeros while startup DMAs run,
            # so the clock gate is at 2.4 GHz when the real matmuls arrive.
            warm_w = warm.tile([P, P], bf16)
            warm_x = warm.tile([P, 256], bf16)
            warm_ps = wpsum.tile([P, 256], f32)
            nc.gpsimd.memset(warm_w[:], 0.0)
            nc.gpsimd.memset(warm_x[:], 0.0)
            for w in range(N_WARM):
                nc.tensor.matmul(
                    warm_ps[:], warm_w[:], warm_x[:],
                    start=(w == 0), stop=(w == N_WARM - 1),
                )

            def w_src(w, it):
                return w.ap()[it].rearrange("p (ko i) -> p ko i", i=P)

            # Startup loads all on the sync HWDGE ring (never SWDGE: its
            # SBUF-resident descriptor rings contend with PE operand
            # streaming), it=0 tiles halved so the first matmuls' deps land
            # earliest in the FIFO. Later weight prefetches issue on sync at
            # the loop bottom: a DMA issue can stall on its tile-slot
            # semaphore, and on the scalar ring that stall would queue ahead
            # of gelu.
            hk = KH // 2
            wg_sbs, wu_sbs = {}, {}
            wg_sbs[0] = wtiles.tile([P, KH, P], bf16, tag="wg", name="wg0")
            wu_sbs[0] = wtiles.tile([P, KH, P], bf16, tag="wu", name="wu0")
            nc.scalar.dma_start(wg_sbs[0][:, :hk], w_src(wg, 0)[:, :hk])
            nc.scalar.dma_start(wg_sbs[0][:, hk:], w_src(wg, 0)[:, hk:])
            nc.sync.dma_start(xt_sb[0][:, :hk], xt.ap()[0][:, :hk])
            nc.sync.dma_start(xt_sb[0][:, hk:], xt.ap()[0][:, hk:])
            nc.sync.dma_start(wu_sbs[0][:, :hk], w_src(wu, 0)[:, :hk])
            nc.sync.dma_start(wu_sbs[0][:, hk:], w_src(wu, 0)[:, hk:])
            # later chunks ride the otherwise-idle scalar ring so the sync
            # FIFO reaches the it>=1 weight tiles ~1.5us sooner
            for c in range(1, NCH):
                nc.scalar.dma_start(xt_sb[c][:], xt.ap()[c])

            PF = 4  # weight prefetch depth (wtiles bufs=6 keeps slots free)

            def _issue_w(it):
                wg_sbs[it] = wtiles.tile([P, KH, P], bf16, tag="wg", name=f"wg{it}")
                nc.sync.dma_start(wg_sbs[it][:], w_src(wg, it))
                wu_sbs[it] = wtiles.tile([P, KH, P], bf16, tag="wu", name=f"wu{it}")
                nc.sync.dma_start(wu_sbs[it][:], w_src(wu, it))

            for it in range(1, min(PF, NI)):
                _issue_w(it)

            # Phase 1: gT/uT = wg^T xT / wu^T xT per i-tile; act = gelu(g)*u.
            # wd tile loads are deferred to the back half of phase 1 (consumed
            # only in phase 2) to keep early bandwidth for wg/wu.
            for it in range(NI):
                wg_sb, wu_sb = wg_sbs.pop(it), wu_sbs.pop(it)
                if it == 2:
                    # probs only feeds phase 2; one-time issue on scalar ring
                    nc.scalar.dma_start(probs_sb[:], probs.ap())


                for c in range(NCH):
                    g_ps = psum.tile([P, MC], f32, tag="g")
                    u_ps = psum.tile([P, MC], f32, tag="u")
                    for k in range(KH):
                        nc.tensor.matmul(
                            g_ps[:],
                            wg_sb[:, k],
                            xt_sb[c][:, k],
                            start=(k == 0),
                            stop=(k == KH - 1),
                        )
                    for k in range(KH):
                        nc.tensor.matmul(
                            u_ps[:],
                            wu_sb[:, k],
                            xt_sb[c][:, k],
                            start=(k == 0),
                            stop=(k == KH - 1),
                        )
                    gt = gtmps.tile([P, MC], f32, tag="gt")
                    if c == 0 and it >= NI - 8:
                        # Gate the two wd row-tile loads of this iteration
                        # behind the phase-1 pipeline: a 1-element read of the
                        # DMA's destination (into gt, fully overwritten by the
                        # gelu below) gives the DMA a WAR dependency on the
                        # loop's pace — otherwise the scheduler hoists all of
                        # wd to t=0 and it steals startup DMA bandwidth.
                        for j in range(2):
                            wd_it = 2 * (it - (NI - 8)) + j
                            nc.vector.tensor_copy(
                                gt[:, j:j + 1], wd_sb[:, wd_it, 0:1]
                            )
                            nc.sync.dma_start(wd_sb[:, wd_it], wd.ap()[wd_it])
                    nc.scalar.activation(
                        gt[:], g_ps[:],
                        mybir.ActivationFunctionType.Gelu_apprx_tanh,
                    )
                    nc.vector.tensor_mul(
                        acts[:, it, c * MC:(c + 1) * MC], gt[:], u_ps[:]
                    )
                if it + PF < NI:
                    _issue_w(it + PF)

            # Phase 2: yT[h, m] = sum_i wd[i, h] * act[i, m], scaled by probs.
            for ht in range(NHT):
                for c in range(NCH):
                    m0 = c * MC
                    y_ps = psum.tile([P, MC], f32, tag="y")
                    for it in range(NI):
                        nc.tensor.matmul(
                            y_ps[:],
                            wd_sb[:, it, ht * P:(ht + 1) * P],
                            acts[:, it, m0:m0 + MC],
                            start=(it == 0),
                            stop=(it == NI - 1),
                        )
                    y_sb = outs.tile([P, MC], f32, tag="y")
                    nc.vector.tensor_mul(y_sb[:], y_ps[:], probs_sb[:, m0:m0 + MC])
                    eng = nc.sync if (ht * NCH + c) % 2 == 0 else nc.scalar
                    eng.dma_start(y.ap()[ht][:, m0:m0 + MC], y_sb[:])

    nc.compile()
    return nc


def _pack_w(w_e):
    """[H, I] bf16 -> [NI, P, KH*P] with w[it, p, k*P+i] = w_e[k*P+p, it*P+i]."""
    w4 = w_e.reshape(KH, P, NI, P)
    return np.ascontiguousarray(w4.transpose(2, 1, 0, 3).reshape(NI, P, KH * P))


def kernel(hidden_states, gate_w, wg, wu, wd):
    global LAST_RESULTS
    x = np.ascontiguousarray(np.asarray(hidden_states, dtype=np.float32))
    gw = np.asarray(gate_w, dtype=np.float32)
    wg = np.asarray(wg, dtype=np.float32)
    wu = np.asarray(wu, dtype=np.float32)
    wd = np.asarray(wd, dtype=np.float32)

    # Router on host (part of the dispatch/sharding step).
    logits = np.tanh((x @ gw) / np.float32(SOFTCAP))
    top2 = np.argsort(-logits, axis=1, kind="stable")[:, :TOPK]  # [T, 2]
    v = np.take_along_axis(logits, top2, axis=1)                 # descending
    ex = np.exp(v - v[:, :1])
    pk = (ex / ex.sum(axis=1, keepdims=True)).astype(np.float32)  # [T, 2]

    token_ids, probs_e = [], []
    for e in range(E):
        mask = top2 == e
        rows = np.where(mask.any(axis=1))[0]
        kk = np.argmax(mask[rows], axis=1)
        token_ids.append(rows)
        probs_e.append(pk[rows, kk])

    n_max = max(len(r) for r in token_ids)
    NCH, MC = _chunk_shape(n_max)
    M_PAD = NCH * MC

    nc = _compiled.get((NCH, MC))
    if nc is None:
        nc = _build(NCH, MC)
        _compiled[(NCH, MC)] = nc

    xb = x.astype(BF16)
    in_maps = []
    for e in range(E):
        ids = token_ids[e]
        xe = np.zeros((M_PAD, H), BF16)
        xe[: len(ids)] = xb[ids]
        # [NCH, MC, KH, P] -> [NCH, P, KH, MC]
        xt_e = np.ascontiguousarray(
            xe.reshape(NCH, MC, KH, P).transpose(0, 3, 2, 1)
        )
        pr = np.zeros((M_PAD,), np.float32)
        pr[: len(ids)] = probs_e[e]
        in_maps.append(
            {
                "xt": xt_e,
                "wg": _pack_w(wg[e].astype(BF16)),
                "wu": _pack_w(wu[e].astype(BF16)),
                "wd": np.ascontiguousarray(wd[e].astype(BF16).reshape(NI, P, H)),
                "probs": np.ascontiguousarray(np.broadcast_to(pr, (P, M_PAD))),
            }
        )

    res = run_bass_kernel_spmd(nc, in_maps, core_ids=list(range(N_CORES)))
    LAST_RESULTS = res

    out = np.zeros((T, H), np.float32)
    for e in range(E):
        ids = token_ids[e]
        # y [NHT, P, M_PAD] -> [M_PAD, H]
        ye = res.results[e]["y"].reshape(H, M_PAD).T
        out[ids] += ye[: len(ids)]
    return out
